# revision 11
# baseline (speedup 1.0000x reference)
"""AttentionalPooler Trainium2 kernel, v2.

Full inputs -> full outputs; data-parallel over batch across 8 NeuronCores
(b=8, one batch element per core).

Per-core math (one batch element):
  xk  = LN(x)                       [4096, 1024]
  q   = (LN(query) @ Wq) * scale    [256, 1024]   (identical on every core)
  kT  = Wk'^T @ xk^T                [1024, 4096]  (K stored transposed)
  V   = xk @ Wv'                    [4096, 1024]  (+ones col per head)
  S^T = kT_h^T-slices @ qT_h        [4096, 256] per head (j on partitions)
  E   = exp(S^T)   (no max subtraction; |S| <= ~7, fp32-safe)
  [O^T_h; den_h] = [V_h | 1]^T @ E  accumulated over j    [65, 256]
  out = sum_h (O_h / den_h) @ Wout_h                      [256, 1024]

v2 structural changes vs v1:
  - x is cast to bf16 host-side; x^T tiles arrive via XBAR DMA-transpose
    (zero PE transposes, zero PSUM-evac copies for the transpose).
  - LN is applied to x^T on the vector engine as a free-dim affine
    (x^T * a + b, a = rstd, b = -mu*rstd); the per-row scalars come from
    bn_stats on a normal-layout copy of x, are transposed once per quarter
    through the PE ([128,16] -> [16,128]), bounced through a DRAM scratch
    and broadcast back with a stride-0-partition DMA.
  - rstd = exp(-0.5*ln(var+eps)) so only one scalar-engine table set
    (natural_log_exp) is ever loaded (no table thrash with attention exp).
  - attention of quarter q is interleaved with the kT projection of the
    same quarter at head-pair granularity; exp latency is hidden behind
    independent projection matmuls; exp is batched 2 PSUM banks per call.
  - a short dummy-matmul burst after the q projection keeps the PE busy
    (and the HAM clock-gate warm) while the first quarter's LN stats and
    weights stream in.

LN gamma and the attention scale are folded into the weights host-side;
LN beta becomes bias vectors applied at PSUM evacuation.
"""

import os
import sys
import types

for _p in ("/root/.axon_site", "/root/.axon_site/_ro/trn_rl_repo", "/opt/trn_rl_repo"):
    if os.path.isdir(_p) and _p not in sys.path:
        sys.path.append(_p)

# The image's antenv package lacks axon_hooks; shim it with the ctypes-based
# NTFF hook from trn_agent_boot so trace=True works under axon.
try:
    import antenv.axon_hooks  # noqa: F401
except ImportError:
    try:
        import trn_agent_boot.trn_boot as _tb

        _hook = _tb._ntff_profile_via_ctypes("/opt/axon/libaxon_pjrt.so")
    except Exception:
        _hook = None
    _m = types.ModuleType("antenv.axon_hooks")
    _m.get_axon_ntff_profile_hook = lambda: _hook
    sys.modules["antenv.axon_hooks"] = _m

import numpy as np

import concourse.bass as bass
import concourse.tile as tile
from concourse import mybir
from concourse.masks import make_identity

KBISECT = os.environ.get("KBISECT", "")

D = 1024          # model dim == ctx dim
NCTX = 4096       # keys per batch element
NQ = 256          # queries
H = 16            # heads
DH = 64           # head dim
NCORES = 8
EPS = 1e-5
QTR = 1024        # keys per quarter
NQTR = NCTX // QTR

F32 = mybir.dt.float32
BF16 = mybir.dt.bfloat16
FP16 = mybir.dt.float16
MM_DT = BF16

Act = mybir.ActivationFunctionType
Alu = mybir.AluOpType


def _patch_drain(max_waits=1):
    """This walrus build rejects >1 sync-wait on the SP Drain that Tile emits
    at kernel exit. Split the waits across a chain of drains."""

    def patched(self, tick_clock, wait_clock):
        from concourse.vector_clock import ScopedClock

        drain_inst = self.nc.sync.drain()
        wait_clock.add_sem_waits(
            drain_inst.ins, ScopedClock({None: tick_clock.global_clock})
        )
        si = drain_inst.ins.sync_info
        waits = list(si.on_wait or []) if si else []
        if len(waits) > max_waits:
            si.on_wait = waits[:max_waits]
            rest = waits[max_waits:]
            while rest:
                extra = self.nc.sync.drain()
                extra.ins.sync_info = mybir.SyncInfo(
                    on_wait=rest[:max_waits], on_update=[]
                )
                rest = rest[max_waits:]
        self.nc.all_engine_barrier()
        assert self.sems is not None
        popped = self.nc._tile_sem_poison_stack.pop()
        assert popped is self._sem_poison
        self.nc.clear_and_free_semaphores(list(self.sems.allocated().values()))
        self.nc.all_engine_barrier()

    tile.TileContext._drain_and_barrier = patched


_patch_drain()


def _split_sync_waits(nc, max_waits=1):
    """This walrus build rejects instructions carrying more than one sync
    wait. Hoist excess waits onto same-engine NoOps placed just before the
    owning instruction (engine queues are serial, so this is equivalent)."""
    for f in nc.m.functions:
        for bb in f.blocks:
            new_list = []
            changed = False
            for inst in bb.instructions:
                si = inst.sync_info
                waits = list(si.on_wait) if si and si.on_wait else []
                if len(waits) > max_waits:
                    changed = True
                    keep = waits[-max_waits:]
                    rest = waits[:-max_waits]
                    k = 0
                    while rest:
                        carrier = mybir.InstNoOp(
                            name=f"{inst.name}-w{k}", ins=[], outs=[]
                        )
                        carrier.engine = inst.engine
                        carrier.sync_info = mybir.SyncInfo(
                            on_wait=rest[:max_waits], on_update=[]
                        )
                        rest = rest[max_waits:]
                        k += 1
                        nc.register_instruction(carrier, overwrite=True)
                        new_list.append(carrier)
                    si.on_wait = keep
                new_list.append(inst)
            if changed:
                bb.instructions = new_list


def build_program():
    nc = bass.Bass("TRN2", target_bir_lowering=False, debug=False)

    x = nc.dram_tensor("x", [NCTX, D], BF16, kind="ExternalInput").ap()
    qry = nc.dram_tensor("qry", [NQ, D], F32, kind="ExternalInput").ap()
    wq = nc.dram_tensor("wq", [D, D], MM_DT, kind="ExternalInput").ap()
    wk = nc.dram_tensor("wk", [D, D], MM_DT, kind="ExternalInput").ap()
    wv = nc.dram_tensor("wv", [D, D], MM_DT, kind="ExternalInput").ap()
    wo = nc.dram_tensor("wo", [D, D], MM_DT, kind="ExternalInput").ap()
    bq = nc.dram_tensor("bq", [128, 8], F32, kind="ExternalInput").ap()
    bk = nc.dram_tensor("bk", [128, 8], F32, kind="ExternalInput").ap()
    bv = nc.dram_tensor("bv", [D], F32, kind="ExternalInput").ap()
    out = nc.dram_tensor("out", [NQ, D], F32, kind="ExternalOutput").ap()

    with tile.TileContext(nc) as tc:
        _build_body(nc, tc, x, qry, wq, wk, wv, wo, bq, bk, bv, out)
    _split_sync_waits(nc)
    return nc


def _build_body(nc, tc, x, qry, wq, wk, wv, wo, bq, bk, bv, out):
    import contextlib

    ctx = contextlib.ExitStack()
    with ctx:
        consts = ctx.enter_context(tc.tile_pool(name="consts", bufs=1))
        wpool = ctx.enter_context(tc.tile_pool(name="wpool", bufs=1))
        qpool = ctx.enter_context(tc.tile_pool(name="qpool", bufs=2))
        xtp = ctx.enter_context(tc.tile_pool(name="xtp", bufs=2))
        xnp = ctx.enter_context(tc.tile_pool(name="xnp", bufs=5))
        ktp = ctx.enter_context(tc.tile_pool(name="ktp", bufs=1))
        vqp = ctx.enter_context(tc.tile_pool(name="vqp", bufs=1))
        statp = ctx.enter_context(tc.tile_pool(name="statp", bufs=2))
        dramp = ctx.enter_context(tc.tile_pool(name="dramp", bufs=2, space="DRAM"))
        etp = ctx.enter_context(tc.tile_pool(name="etp", bufs=4))
        big = ctx.enter_context(tc.tile_pool(name="big", bufs=1))
        outp = ctx.enter_context(tc.tile_pool(name="outp", bufs=1))
        ps_mm = ctx.enter_context(tc.tile_pool(name="ps_mm", bufs=2, space="PSUM"))
        ps_st = ctx.enter_context(tc.tile_pool(name="ps_st", bufs=2, space="PSUM"))
        ps_ot = ctx.enter_context(tc.tile_pool(name="ps_ot", bufs=2, space="PSUM"))

        # ---- constants ----
        identb = consts.tile([128, 128], MM_DT, tag="identb")
        make_identity(nc, identb)
        identh = consts.tile([128, 128], FP16, tag="identh")
        make_identity(nc, identh)
        eps_t = consts.tile([128, 1], F32, tag="eps")
        nc.vector.memset(eps_t, EPS)
        ones_t = consts.tile([128, 64], F32, tag="ones")
        nc.vector.memset(ones_t, 1.0)
        bq_sb = consts.tile([128, 8], F32, tag="bq")
        nc.sync.dma_start(out=bq_sb, in_=bq)
        bk_sb = consts.tile([128, 8], F32, tag="bk")
        nc.sync.dma_start(out=bk_sb, in_=bk)
        bv_rep = consts.tile([128, D], BF16, tag="bvrep")
        bv_bcast = bass.AP(tensor=bv.tensor, offset=bv.offset,
                           ap=[[0, 128]] + list(bv.ap))
        nc.gpsimd.dma_start(out=bv_rep, in_=bv_bcast)

        # ---- weight streams (ACT queue). wq first (q proj is the first PE
        # work), wv before wk (V phase precedes kT phase in each quarter) ----
        wq_r = wq.rearrange("(c p) e -> p c e", p=128)
        wv_r = wv.rearrange("(c p) e -> p c e", p=128)
        wk_r = wk.rearrange("(c p) e -> p c e", p=128)
        wq_sb = vqp.tile([128, 8, 8, 128], MM_DT, tag="vq")
        for ec in range(8):
            nc.scalar.dma_start(out=wq_sb[:, ec, :, :],
                                in_=wq_r[:, :, ec * 128:(ec + 1) * 128])
        wv_sb = wpool.tile([128, 8, D], MM_DT, tag="wv")
        for dc in range(8):
            nc.scalar.dma_start(out=wv_sb[:, dc, :], in_=wv_r[:, dc, :])
        wk_sb = wpool.tile([128, 8, D], MM_DT, tag="wk")
        for dc in range(8):
            nc.scalar.dma_start(out=wk_sb[:, dc, :], in_=wk_r[:, dc, :])
        wo_sb = wpool.tile([64, H, D], MM_DT, tag="wo")
        nc.scalar.dma_start(out=wo_sb, in_=wo.rearrange("(h p) f -> p h f", p=64))

        # ---- per-quarter input DMAs ----
        xt_tiles = [None] * NQTR     # [128, 8, QTR] x^T quarters (bf16)
        xn_tiles = [[None] * 8 for _ in range(NQTR)]

        def emit_input_dmas(q):
            if q >= NQTR:
                return
            xt = xtp.tile([128, 8, QTR], BF16, tag="xT")
            for dc in range(8):
                nc.sync.dma_start(
                    out=xt[:, dc, :],
                    in_=x[q * QTR:(q + 1) * QTR, dc * 128:(dc + 1) * 128],
                    transpose=True,
                )
            xt_tiles[q] = xt
            xn_row = xn_tiles[q]
            for jt in range(8):
                j0 = q * QTR + jt * 128
                t = xnp.tile([128, D], BF16, tag="xn")
                nc.sync.dma_start(out=t, in_=x[j0:j0 + 128, :])
                xn_row[jt] = t

        # ---- per-quarter LN stats -> a/b replicated tiles ----
        reps = [None] * NQTR  # (arep, brep) fp16 [128, 8, 128]

        def emit_stats(q):
            if q >= NQTR or KBISECT == "nostats":
                return
            stats = statp.tile([128, 8, 2, nc.vector.BN_STATS_DIM], F32, tag="bst")
            mvq = statp.tile([128, 8, 2], F32, tag="mvq")
            for jt in range(8):
                t = xn_tiles[q][jt]
                for sg in range(2):
                    nc.vector.bn_stats(
                        out=stats[:, jt, sg, :], in_=t[:, sg * 512:(sg + 1) * 512]
                    )
                nc.vector.bn_aggr(out=mvq[:, jt, :], in_=stats[:, jt, :, :])
            lnv = statp.tile([128, 8], F32, tag="lnv")
            nc.scalar.activation(out=lnv, in_=mvq[:, :, 1:2],
                                 func=Act.Ln, bias=eps_t)
            pack = statp.tile([128, 16], FP16, tag="pack")
            nc.scalar.activation(out=pack[:, 0:8], in_=lnv,
                                 func=Act.Exp, scale=-0.5)
            nc.vector.scalar_tensor_tensor(
                out=pack[:, 8:16], in0=pack[:, 0:8], scalar=-1.0,
                in1=mvq[:, :, 0:1], op0=Alu.mult, op1=Alu.mult,
            )
            ptp = ps_st.tile([16, 128], FP16, tag="st")
            nc.tensor.transpose(ptp, pack, identh)
            pt_sb = statp.tile([16, 128], FP16, tag="pt")
            nc.vector.tensor_copy(out=pt_sb, in_=ptp)
            # NB: must NOT go on the sync queue — xn loads can stall there
            # waiting for bn_stats buffer frees, and normalize (behind
            # bn_stats on the DVE queue) waits on the broadcast below.
            pt_d = dramp.tile([16, 128], FP16, tag="ptd")
            nc.gpsimd.dma_start(out=pt_d, in_=pt_sb)
            arep = statp.tile([128, 8, 128], FP16, tag="arep")
            brep = statp.tile([128, 8, 128], FP16, tag="brep")
            for half, dst in ((0, arep), (1, brep)):
                src = pt_d[half * 8:(half + 1) * 8, :]
                nc.gpsimd.dma_start(
                    out=dst,
                    in_=bass.AP(tensor=src.tensor, offset=src.offset,
                                ap=[[0, 128]] + list(src.ap)),
                )
            reps[q] = (arep, brep)

        def emit_normalize(q):
            if q >= NQTR or KBISECT == "nostats":
                return
            arep, brep = reps[q]
            af = arep.rearrange("p c f -> p (c f)")
            bf = brep.rearrange("p c f -> p (c f)")
            xt = xt_tiles[q]
            for dc in range(8):
                nc.vector.tensor_tensor(out=xt[:, dc, :], in0=xt[:, dc, :],
                                        in1=af, op=Alu.mult)
                nc.vector.tensor_tensor(out=xt[:, dc, :], in0=xt[:, dc, :],
                                        in1=bf, op=Alu.add)

        # ---- q path: qT = (LN(query) @ Wq)^T + bq, stored [e', ec, i] ----
        qT = consts.tile([128, 8, NQ], MM_DT, tag="qT")
        qts = []
        for t in range(2):
            qt = qpool.tile([128, D], F32, tag="xt")
            nc.sync.dma_start(out=qt, in_=qry[t * 128:(t + 1) * 128, :])
            qts.append(qt)

        # first-quarter input DMAs go out right behind qry
        emit_input_dmas(0)
        emit_input_dmas(1)

        qnT = consts.tile([128, 8, NQ], MM_DT, tag="qnT")
        for t in range(2):
            qt = qts[t]
            stats = qpool.tile([128, 2, nc.vector.BN_STATS_DIM], F32, tag="qst")
            for sg in range(2):
                nc.vector.bn_stats(out=stats[:, sg, :],
                                   in_=qt[:, sg * 512:(sg + 1) * 512])
            mv = qpool.tile([128, 2], F32, tag="qmv")
            nc.vector.bn_aggr(out=mv, in_=stats)
            lnv = qpool.tile([128, 1], F32, tag="qlnv")
            nc.scalar.activation(out=lnv, in_=mv[:, 1:2], func=Act.Ln, bias=eps_t)
            rstd = qpool.tile([128, 1], F32, tag="qrstd")
            nc.scalar.activation(out=rstd, in_=lnv, func=Act.Exp, scale=-0.5)
            qnb = qpool.tile([128, D], MM_DT, tag="qnb")
            nc.vector.tensor_scalar(
                out=qnb, in0=qt, scalar1=mv[:, 0:1], scalar2=rstd,
                op0=Alu.subtract, op1=Alu.mult,
            )
            for c in range(4):
                ptr = ps_st.tile([128, 2, 128], MM_DT, tag="st")
                for k in range(2):
                    dc = c * 2 + k
                    nc.tensor.transpose(
                        ptr[:, k, :], qnb[:, dc * 128:(dc + 1) * 128], identb
                    )
                nc.vector.tensor_copy(
                    out=qnT[:, c * 2:c * 2 + 2, t * 128:(t + 1) * 128], in_=ptr
                )
        for ec in range(8):
            psq = ps_mm.tile([128, NQ], F32, tag="mm")
            for dc in range(8):
                nc.tensor.matmul(
                    psq, lhsT=wq_sb[:, ec, dc, :], rhs=qnT[:, dc, :],
                    start=(dc == 0), stop=(dc == 7),
                )
            nc.vector.tensor_scalar(
                out=qT[:, ec, :], in0=psq, scalar1=bq_sb[:, ec:ec + 1],
                scalar2=None, op0=Alu.add,
            )

        # stats for quarter 0 as soon as its xn tiles land
        emit_stats(0)

        # dummy warm matmuls: keep PE busy (HAM warm) while quarter-0 LN
        # stats and the kv weights stream in
        warm_ps = ps_ot.tile([128, 512], F32, tag="ot")  # 1 bank
        for i in range(12):
            nc.tensor.matmul(warm_ps, lhsT=identb, rhs=qnT[:, 0:2, :],
                             start=True, stop=True)

        emit_normalize(0)

        # accumulators: [O^T_h ; den_h] per head
        otacc = big.tile([65, H, NQ], F32, tag="ot")
        ot_n = big.tile([64, H, NQ], MM_DT, tag="otn")

        def emit_v_proj(q):
            v_q = vqp.tile([128, 8, H * 65], MM_DT, tag="vq")
            xt = xt_tiles[q]
            for jt in range(8):
                for nt in range(2):
                    psv = ps_mm.tile([128, 512], F32, tag="mm")
                    for dc in range(8):
                        nc.tensor.matmul(
                            psv,
                            lhsT=xt[:, dc, jt * 128:(jt + 1) * 128],
                            rhs=wv_sb[:, dc, nt * 512:(nt + 1) * 512],
                            start=(dc == 0), stop=(dc == 7),
                        )
                    vdst = v_q[:, jt, nt * 8 * 65:(nt + 1) * 8 * 65].rearrange(
                        "p (h c) -> p h c", c=65
                    )[:, :, 0:64]
                    nc.vector.tensor_add(
                        out=vdst,
                        in0=psv.rearrange("p (h c) -> p h c", c=64),
                        in1=bv_rep[:, nt * 512:(nt + 1) * 512].rearrange(
                            "p (h c) -> p h c", c=64
                        ),
                    )
            ones_view = v_q.rearrange("p j (h c) -> p j h c", c=65)[:, :, :, 64:65]
            nc.vector.memset(ones_view, 1.0)
            return v_q

        def emit_kt_slice(kT_q, q, ec):
            xt = xt_tiles[q]
            for half in range(2):
                psk = ps_mm.tile([128, 512], F32, tag="mm")
                for dc in range(8):
                    nc.tensor.matmul(
                        psk,
                        lhsT=wk_sb[:, dc, ec * 128:(ec + 1) * 128],
                        rhs=xt[:, dc, half * 512:(half + 1) * 512],
                        start=(dc == 0), stop=(dc == 7),
                    )
                nc.vector.tensor_scalar(
                    out=kT_q[:, ec, half * 512:(half + 1) * 512], in0=psk,
                    scalar1=bk_sb[:, ec:ec + 1], scalar2=None, op0=Alu.add,
                )

        def emit_scores_exp(kT_q, hc):
            ets = []
            for jjp in range(4):
                pst = ps_st.tile([128, 4, NQ], F32, tag="st")
                # slot = par*2 + u: each par (concurrent row-group MMs)
                # gets its own PSUM bank; within a bank the two u-groups
                # run sequentially
                for u in range(2):
                    jj = jjp * 2 + u
                    for par in range(2):
                        pb = par * 64
                        nc.tensor.matmul(
                            pst[:, par * 2 + u, :],
                            lhsT=kT_q[pb:pb + 64, hc, jj * 128:(jj + 1) * 128],
                            rhs=qT[pb:pb + 64, hc, :],
                            start=True, stop=True,
                        )
                et = etp.tile([128, 4, NQ], MM_DT, tag="et")
                nc.scalar.activation(out=et, in_=pst, func=Act.Exp)
                ets.append(et)
            return ets

        def emit_attnv(v_q, ets, hc, q):
            # separate bank-padded tiles per head: an accumulation group's
            # start=True zeroes its whole PSUM bank
            pso0 = ps_ot.tile([65, 512], F32, tag="ot")
            pso1 = ps_ot.tile([65, 512], F32, tag="ot")
            psos = (pso0, pso1)
            for jjp in range(4):
                for u in range(2):
                    jj = jjp * 2 + u
                    for par in range(2):
                        h = hc * 2 + par
                        nc.tensor.matmul(
                            psos[par][:, 0:NQ],
                            lhsT=v_q[:, jj, h * 65:(h + 1) * 65],
                            rhs=ets[jjp][:, par * 2 + u, :],
                            start=(jj == 0), stop=(jj == 7),
                        )
            for par in range(2):
                h = 2 * hc + par
                if q == 0:
                    nc.vector.tensor_copy(out=otacc[:, h, :],
                                          in_=psos[par][:, 0:NQ])
                else:
                    nc.vector.tensor_add(out=otacc[:, h, :],
                                         in0=otacc[:, h, :],
                                         in1=psos[par][:, 0:NQ])

        def emit_head_norm(hc):
            # O_h /= den_h for heads 2hc, 2hc+1 (den kept in otacc row 64)
            nc.vector.reciprocal(
                out=otacc[64:65, 2 * hc:2 * hc + 2, :],
                in_=otacc[64:65, 2 * hc:2 * hc + 2, :],
            )
            for k in range(2):
                h = 2 * hc + k
                psb = ps_st.tile([64, NQ], F32, tag="st")
                nc.tensor.matmul(
                    psb, lhsT=ones_t[64:65, :], rhs=otacc[64:65, h, :],
                    start=True, stop=True,
                )
                nc.vector.tensor_mul(
                    out=ot_n[:, h, :], in0=otacc[0:64, h, :], in1=psb
                )

        # ---- main quarter loop ----
        for q in range(NQTR):
            emit_input_dmas(q + 2)
            v_q = emit_v_proj(q)
            kT_q = ktp.tile([128, 8, QTR], MM_DT, tag="kt")
            pending = None  # (ets, hc) awaiting attnv
            for hc in range(8):
                emit_kt_slice(kT_q, q, hc)
                if KBISECT != "noattn":
                    ets = emit_scores_exp(kT_q, hc)
                    if pending is not None:
                        emit_attnv(v_q, pending[0], pending[1], q)
                        if q == NQTR - 1:
                            emit_head_norm(pending[1])
                    pending = (ets, hc)
                if hc == 2:
                    emit_stats(q + 1)
                if hc == 5:
                    emit_normalize(q + 1)
            if KBISECT != "noattn":
                emit_attnv(v_q, pending[0], pending[1], q)
                if q == NQTR - 1:
                    emit_head_norm(pending[1])

        # ---- out = sum_h O_h @ Wout_h ----
        if KBISECT == "noattn":
            zt = outp.tile([128, D], F32, tag="outsb")
            nc.vector.memset(zt, 0.0)
            nc.vector.tensor_add(out=zt[:, 0:1], in0=kT_q[:, 0, 0:1],
                                 in1=v_q[:, 0, 0:1])
            for ic in range(2):
                nc.sync.dma_start(out=out[ic * 128:(ic + 1) * 128, :], in_=zt)
            return
        for ic in range(2):
            psf0 = ps_mm.tile([128, 512], F32, tag="mm")
            psf1 = ps_mm.tile([128, 512], F32, tag="mm")
            psf = [psf0, psf1]
            for h in range(16):
                for ft in range(2):
                    nc.tensor.matmul(
                        psf[ft],
                        lhsT=ot_n[:, h, ic * 128:(ic + 1) * 128],
                        rhs=wo_sb[:, h, ft * 512:(ft + 1) * 512],
                        start=(h == 0), stop=(h == 15),
                    )
            ot = outp.tile([128, D], F32, tag="outsb")
            for ft in range(2):
                nc.scalar.activation(
                    out=ot[:, ft * 512:(ft + 1) * 512], in_=psf[ft],
                    func=Act.Copy,
                )
            nc.sync.dma_start(out=out[ic * 128:(ic + 1) * 128, :], in_=ot)


_CACHED = None


def _get_program():
    global _CACHED
    if _CACHED is None:
        _CACHED = build_program()
    return _CACHED


def _mm_np():
    import ml_dtypes

    return ml_dtypes.bfloat16


def _prep_inputs(x, query, Wq, Wkv, Wout, ln_q_g, ln_q_b, ln_k_g, ln_k_b):
    scale = DH ** -0.5
    f32 = np.float32
    Wq = np.asarray(Wq, f32)
    Wkv = np.asarray(Wkv, f32)
    Wout = np.asarray(Wout, f32)
    wq_eff = (np.asarray(ln_q_g, f32)[:, None] * Wq * scale).astype(f32)
    bq_eff = (np.asarray(ln_q_b, f32) @ Wq * scale).astype(f32)
    wk_eff = (np.asarray(ln_k_g, f32)[:, None] * Wkv[:, :D]).astype(f32)
    bk_eff = (np.asarray(ln_k_b, f32) @ Wkv[:, :D]).astype(f32)
    wv_eff = (np.asarray(ln_k_g, f32)[:, None] * Wkv[:, D:]).astype(f32)
    bv_eff = (np.asarray(ln_k_b, f32) @ Wkv[:, D:]).astype(f32)
    mdt = _mm_np()
    shared = {
        "qry": np.ascontiguousarray(np.asarray(query, f32)),
        "wq": np.ascontiguousarray(wq_eff.astype(mdt)),
        "wk": np.ascontiguousarray(wk_eff.astype(mdt)),
        "wv": np.ascontiguousarray(wv_eff.astype(mdt)),
        "wo": np.ascontiguousarray(Wout.astype(mdt)),
        "bq": np.ascontiguousarray(bq_eff.reshape(8, 128).T),
        "bk": np.ascontiguousarray(bk_eff.reshape(8, 128).T),
        "bv": np.ascontiguousarray(bv_eff),
    }
    x = np.asarray(x, f32).astype(mdt)
    in_maps = [
        dict(shared, x=np.ascontiguousarray(x[i])) for i in range(NCORES)
    ]
    return in_maps


def run(trace=False, **inputs):
    from concourse.bass_utils import run_bass_kernel_spmd

    nc = _get_program()
    in_maps = _prep_inputs(**inputs)
    res = run_bass_kernel_spmd(
        nc, in_maps, core_ids=list(range(NCORES)), trace=trace
    )
    out = np.stack([res.results[i]["out"] for i in range(NCORES)], axis=0)
    return out.astype(np.float32), res.exec_time_ns


def kernel(**inputs):
    out, _ = run(trace=False, **inputs)
    return out


# revision 15
# speedup vs baseline: 1.2761x; 1.2761x over previous
"""AttentionalPooler Trainium2 kernel, v3.

Full inputs -> full outputs; data-parallel over batch across 8 NeuronCores
(b=8, one batch element per core).

Per-core math (one batch element):
  xk  = LN(x)                       [4096, 1024]
  q   = (LN(query) @ Wq) * scale    [256, 1024]   (identical on every core)
  kT  = Wk'^T @ xk^T                [1024, 4096]  (K stored transposed)
  V   = xk @ Wv'                    [4096, 1024]  (+ones col per head)
  S^T = kT_h^T-slices @ qT_h        [4096, 256] per head (j on partitions)
  E   = exp(S^T)   (no max subtraction; |S| <= ~7, fp32-safe)
  [O^T_h; den_h] = [V_h | 1]^T @ E  accumulated over j    [65, 256]
  out = sum_h (O_h / den_h) @ Wout_h                      [256, 1024]

Pipeline (vs the v1 baseline):
  - x is cast to bf16 host-side (halves x DMA traffic).
  - per-quarter batched LN stats; rstd = exp(-0.5*ln(var+eps)) on the
    scalar engine so only one activation table set (natural_log_exp) is
    ever loaded (no table thrash against attention's exp).
  - LN normalize is applied in place (per-partition affine), then x^T is
    built with PE transposes; the xk^T quarter for q+1 is produced while
    quarter q computes, interleaved at head-pair granularity.
  - attention of quarter q runs inside quarter q, interleaved with the kT
    projection slices; exp is batched 1024 elems/lane over 2 PSUM banks;
    concurrent row-group score matmuls write separate PSUM banks (a
    start=True in a bank an in-flight matmul is draining into hangs HW).
  - a short dummy-matmul burst after the q projection keeps the PE (and
    the HAM clock-gate) warm while weights and the first quarter stream.

LN gamma and the attention scale are folded into the weights host-side;
LN beta becomes bias vectors applied at PSUM evacuation.
"""

import os
import sys
import types

for _p in ("/root/.axon_site", "/root/.axon_site/_ro/trn_rl_repo", "/opt/trn_rl_repo"):
    if os.path.isdir(_p) and _p not in sys.path:
        sys.path.append(_p)

# The image's antenv package lacks axon_hooks; shim it with the ctypes-based
# NTFF hook from trn_agent_boot so trace=True works under axon.
try:
    import antenv.axon_hooks  # noqa: F401
except ImportError:
    try:
        import trn_agent_boot.trn_boot as _tb

        _hook = _tb._ntff_profile_via_ctypes("/opt/axon/libaxon_pjrt.so")
    except Exception:
        _hook = None
    _m = types.ModuleType("antenv.axon_hooks")
    _m.get_axon_ntff_profile_hook = lambda: _hook
    sys.modules["antenv.axon_hooks"] = _m

import numpy as np

import concourse.bass as bass
import concourse.tile as tile
from concourse import mybir
from concourse.masks import make_identity

D = 1024          # model dim == ctx dim
NCTX = 4096       # keys per batch element
NQ = 256          # queries
H = 16            # heads
DH = 64           # head dim
NCORES = 8
EPS = 1e-5
QTR = 1024        # keys per quarter
NQTR = NCTX // QTR

F32 = mybir.dt.float32
BF16 = mybir.dt.bfloat16
MM_DT = BF16

Act = mybir.ActivationFunctionType
Alu = mybir.AluOpType


def _patch_drain(max_waits=1):
    """This walrus build rejects >1 sync-wait on the SP Drain that Tile emits
    at kernel exit. Split the waits across a chain of drains."""

    def patched(self, tick_clock, wait_clock):
        from concourse.vector_clock import ScopedClock

        drain_inst = self.nc.sync.drain()
        wait_clock.add_sem_waits(
            drain_inst.ins, ScopedClock({None: tick_clock.global_clock})
        )
        si = drain_inst.ins.sync_info
        waits = list(si.on_wait or []) if si else []
        if len(waits) > max_waits:
            si.on_wait = waits[:max_waits]
            rest = waits[max_waits:]
            while rest:
                extra = self.nc.sync.drain()
                extra.ins.sync_info = mybir.SyncInfo(
                    on_wait=rest[:max_waits], on_update=[]
                )
                rest = rest[max_waits:]
        self.nc.all_engine_barrier()
        assert self.sems is not None
        popped = self.nc._tile_sem_poison_stack.pop()
        assert popped is self._sem_poison
        self.nc.clear_and_free_semaphores(list(self.sems.allocated().values()))
        self.nc.all_engine_barrier()

    tile.TileContext._drain_and_barrier = patched


_patch_drain()


def _split_sync_waits(nc, max_waits=1):
    """This walrus build rejects instructions carrying more than one sync
    wait. Hoist excess waits onto same-engine NoOps placed just before the
    owning instruction (engine queues are serial, so this is equivalent)."""
    for f in nc.m.functions:
        for bb in f.blocks:
            new_list = []
            changed = False
            for inst in bb.instructions:
                si = inst.sync_info
                waits = list(si.on_wait) if si and si.on_wait else []
                if len(waits) > max_waits:
                    changed = True
                    keep = waits[-max_waits:]
                    rest = waits[:-max_waits]
                    k = 0
                    while rest:
                        carrier = mybir.InstNoOp(
                            name=f"{inst.name}-w{k}", ins=[], outs=[]
                        )
                        carrier.engine = inst.engine
                        carrier.sync_info = mybir.SyncInfo(
                            on_wait=rest[:max_waits], on_update=[]
                        )
                        rest = rest[max_waits:]
                        k += 1
                        nc.register_instruction(carrier, overwrite=True)
                        new_list.append(carrier)
                    si.on_wait = keep
                new_list.append(inst)
            if changed:
                bb.instructions = new_list


def build_program():
    nc = bass.Bass("TRN2", target_bir_lowering=False, debug=False)

    x = nc.dram_tensor("x", [NCTX, D], BF16, kind="ExternalInput").ap()
    qry = nc.dram_tensor("qry", [NQ, D], F32, kind="ExternalInput").ap()
    wq = nc.dram_tensor("wq", [D, D], MM_DT, kind="ExternalInput").ap()
    wk = nc.dram_tensor("wk", [D, D], MM_DT, kind="ExternalInput").ap()
    wv = nc.dram_tensor("wv", [D, D], MM_DT, kind="ExternalInput").ap()
    wo = nc.dram_tensor("wo", [D, D], MM_DT, kind="ExternalInput").ap()
    bq = nc.dram_tensor("bq", [128, 8], F32, kind="ExternalInput").ap()
    bk = nc.dram_tensor("bk", [128, 8], F32, kind="ExternalInput").ap()
    bv = nc.dram_tensor("bv", [D], F32, kind="ExternalInput").ap()
    out = nc.dram_tensor("out", [NQ, D], F32, kind="ExternalOutput").ap()

    with tile.TileContext(nc) as tc:
        _build_body(nc, tc, x, qry, wq, wk, wv, wo, bq, bk, bv, out)
    _split_sync_waits(nc)
    return nc


def _build_body(nc, tc, x, qry, wq, wk, wv, wo, bq, bk, bv, out):
    import contextlib

    ctx = contextlib.ExitStack()
    with ctx:
        consts = ctx.enter_context(tc.tile_pool(name="consts", bufs=1))
        wpool = ctx.enter_context(tc.tile_pool(name="wpool", bufs=1))
        qpool = ctx.enter_context(tc.tile_pool(name="qpool", bufs=2))
        xtp = ctx.enter_context(tc.tile_pool(name="xtp", bufs=2))
        xnp = ctx.enter_context(tc.tile_pool(name="xnp", bufs=8))
        ktp = ctx.enter_context(tc.tile_pool(name="ktp", bufs=1))
        vqp = ctx.enter_context(tc.tile_pool(name="vqp", bufs=1))
        statp = ctx.enter_context(tc.tile_pool(name="statp", bufs=2))
        etp = ctx.enter_context(tc.tile_pool(name="etp", bufs=4))
        big = ctx.enter_context(tc.tile_pool(name="big", bufs=1))
        outp = ctx.enter_context(tc.tile_pool(name="outp", bufs=1))
        ps_mm = ctx.enter_context(tc.tile_pool(name="ps_mm", bufs=2, space="PSUM"))
        ps_st = ctx.enter_context(tc.tile_pool(name="ps_st", bufs=2, space="PSUM"))
        ps_ot = ctx.enter_context(tc.tile_pool(name="ps_ot", bufs=2, space="PSUM"))

        # ---- constants ----
        identb = consts.tile([128, 128], MM_DT, tag="identb")
        make_identity(nc, identb)
        eps_t = consts.tile([128, 1], F32, tag="eps")
        nc.vector.memset(eps_t, EPS)
        ones_t = consts.tile([128, 64], F32, tag="ones")
        nc.vector.memset(ones_t, 1.0)
        bq_sb = consts.tile([128, 8], F32, tag="bq")
        nc.sync.dma_start(out=bq_sb, in_=bq)
        bk_sb = consts.tile([128, 8], F32, tag="bk")
        nc.sync.dma_start(out=bk_sb, in_=bk)
        bv_rep = consts.tile([128, D], BF16, tag="bvrep")
        bv_bcast = bass.AP(tensor=bv.tensor, offset=bv.offset,
                           ap=[[0, 128]] + list(bv.ap))
        nc.gpsimd.dma_start(out=bv_rep, in_=bv_bcast)

        # ---- q path, part A: load + LN stats (puts the Ln/Exp table load
        # at the very front of the ACT queue) ----
        qT = consts.tile([128, 8, NQ], MM_DT, tag="qT")
        qnT = consts.tile([128, 8, NQ], MM_DT, tag="qnT")
        qts, qmvs = [], []
        for t in range(2):
            qt = qpool.tile([128, D], F32, tag="qt")
            nc.sync.dma_start(out=qt, in_=qry[t * 128:(t + 1) * 128, :])
            qts.append(qt)
        for t in range(2):
            qt = qts[t]
            stats = qpool.tile([128, 2, nc.vector.BN_STATS_DIM], F32, tag="qst")
            for sg in range(2):
                nc.vector.bn_stats(out=stats[:, sg, :],
                                   in_=qt[:, sg * 512:(sg + 1) * 512])
            mv = qpool.tile([128, 2], F32, tag="qmv")
            nc.vector.bn_aggr(out=mv, in_=stats)
            lnv = qpool.tile([128, 1], F32, tag="qlnv")
            nc.scalar.activation(out=lnv, in_=mv[:, 1:2], func=Act.Ln, bias=eps_t)
            rstd = qpool.tile([128, 1], F32, tag="qrstd")
            nc.scalar.activation(out=rstd, in_=lnv, func=Act.Exp, scale=-0.5)
            qmvs.append((mv, rstd))

        # ---- weight streams (ACT queue, few big DMAs). wq first (q proj is
        # the first real PE work), wv before wk (V precedes kT per quarter),
        # wo last (needed only at the tail) ----
        wq_r = wq.rearrange("(c p) e -> p c e", p=128)
        wq_sb = vqp.tile([128, 8, 8, 128], MM_DT, tag="vq")
        for ec in range(8):
            nc.scalar.dma_start(out=wq_sb[:, ec, :, :],
                                in_=wq_r[:, :, ec * 128:(ec + 1) * 128])
        wv_r = wv.rearrange("(c p) e -> p c e", p=128)
        wv_sb = wpool.tile([128, 8, D], MM_DT, tag="wv")
        for h2 in range(2):
            nc.scalar.dma_start(out=wv_sb[:, h2 * 4:(h2 + 1) * 4, :],
                                in_=wv_r[:, h2 * 4:(h2 + 1) * 4, :])
        wk_r = wk.rearrange("(c p) e -> p c e", p=128)
        wk_sb = wpool.tile([128, 8, D], MM_DT, tag="wk")
        for h2 in range(2):
            nc.scalar.dma_start(out=wk_sb[:, h2 * 4:(h2 + 1) * 4, :],
                                in_=wk_r[:, h2 * 4:(h2 + 1) * 4, :])
        wo_sb = wpool.tile([64, H, D], MM_DT, tag="wo")
        nc.scalar.dma_start(out=wo_sb, in_=wo.rearrange("(h p) f -> p h f", p=64))

        # ---- per-quarter x loads (bf16, normal layout; the only x read) ----
        xn_tiles = [[None] * 8 for _ in range(NQTR)]

        def emit_input_dmas(q):
            if q >= NQTR:
                return
            for jt in range(8):
                j0 = q * QTR + jt * 128
                t = xnp.tile([128, D], BF16, tag="xn")
                nc.sync.dma_start(out=t, in_=x[j0:j0 + 128, :])
                xn_tiles[q][jt] = t

        # ---- per-quarter LN stats: mu, rstd for 8 tiles ----
        stat_res = [None] * NQTR  # (mvq, rstd_q)

        def emit_stats(q):
            if q >= NQTR:
                return
            stats = statp.tile([128, 8, 2, nc.vector.BN_STATS_DIM], F32, tag="bst")
            mvq = statp.tile([128, 8, 2], F32, tag="mvq")
            for jt in range(8):
                t = xn_tiles[q][jt]
                for sg in range(2):
                    nc.vector.bn_stats(
                        out=stats[:, jt, sg, :], in_=t[:, sg * 512:(sg + 1) * 512]
                    )
                nc.vector.bn_aggr(out=mvq[:, jt, :], in_=stats[:, jt, :, :])
            lnv = statp.tile([128, 8], F32, tag="lnv")
            nc.scalar.activation(out=lnv, in_=mvq[:, :, 1:2],
                                 func=Act.Ln, bias=eps_t)
            rstd_q = statp.tile([128, 8], F32, tag="rstdq")
            nc.scalar.activation(out=rstd_q, in_=lnv, func=Act.Exp, scale=-0.5)
            stat_res[q] = (mvq, rstd_q)

        # xk^T quarters, built one quarter ahead
        xkt_tiles = [None] * NQTR

        def alloc_xkt(q):
            if q >= NQTR:
                return
            xkt_tiles[q] = xtp.tile([128, 8, QTR], MM_DT, tag="xT",
                                    name=f"xkt{q}")

        def emit_prep_tile(q, jt):
            """LN-normalize x tile jt of quarter q in place, then transpose
            it into xkt_tiles[q][:, :, jt*128:(jt+1)*128]."""
            if q >= NQTR:
                return
            mvq, rstd_q = stat_res[q]
            t = xn_tiles[q][jt]
            nc.vector.tensor_scalar(
                out=t, in0=t, scalar1=mvq[:, jt, 0:1], scalar2=rstd_q[:, jt:jt + 1],
                op0=Alu.subtract, op1=Alu.mult,
            )
            xkt = xkt_tiles[q]
            for c in range(4):
                ptr = ps_st.tile([128, 2, 128], MM_DT, tag="st")
                for k in range(2):
                    dc = c * 2 + k
                    nc.tensor.transpose(
                        ptr[:, k, :], t[:, dc * 128:(dc + 1) * 128], identb
                    )
                nc.vector.tensor_copy(
                    out=xkt[:, c * 2:c * 2 + 2, jt * 128:(jt + 1) * 128], in_=ptr
                )

        # ---- q path, part B: normalize, transpose, project ----
        emit_input_dmas(0)
        emit_input_dmas(1)

        for t in range(2):
            qt = qts[t]
            mv, rstd = qmvs[t]
            qnb = qpool.tile([128, D], MM_DT, tag="qnb")
            nc.vector.tensor_scalar(
                out=qnb, in0=qt, scalar1=mv[:, 0:1], scalar2=rstd,
                op0=Alu.subtract, op1=Alu.mult,
            )
            for c in range(4):
                ptr = ps_st.tile([128, 2, 128], MM_DT, tag="st")
                for k in range(2):
                    dc = c * 2 + k
                    nc.tensor.transpose(
                        ptr[:, k, :], qnb[:, dc * 128:(dc + 1) * 128], identb
                    )
                nc.vector.tensor_copy(
                    out=qnT[:, c * 2:c * 2 + 2, t * 128:(t + 1) * 128], in_=ptr
                )
        for ec in range(8):
            psq = ps_mm.tile([128, NQ], F32, tag="mm")
            for dc in range(8):
                nc.tensor.matmul(
                    psq, lhsT=wq_sb[:, ec, dc, :], rhs=qnT[:, dc, :],
                    start=(dc == 0), stop=(dc == 7),
                )
            nc.vector.tensor_scalar(
                out=qT[:, ec, :], in0=psq, scalar1=bq_sb[:, ec:ec + 1],
                scalar2=None, op0=Alu.add,
            )

        # quarter 0 prep: stats, then dummy warm matmuls keep the PE busy
        # (HAM warm) while stats/weights stream, then normalize+transpose
        emit_stats(0)
        warm_ps = ps_ot.tile([128, 512], F32, tag="ot")
        for i in range(8):
            nc.tensor.matmul(warm_ps, lhsT=identb, rhs=qnT[:, 0:2, :],
                             start=True, stop=True)
        alloc_xkt(0)
        for jt in range(8):
            emit_prep_tile(0, jt)

        # accumulators: [O^T_h ; den_h] per head
        otacc = big.tile([65, H, NQ], F32, tag="ot")
        ot_n = big.tile([64, H, NQ], MM_DT, tag="otn")

        def emit_v_proj(q):
            v_q = vqp.tile([128, 8, H * 65], MM_DT, tag="vq")
            xkt = xkt_tiles[q]
            for jt in range(8):
                for nt in range(2):
                    psv = ps_mm.tile([128, 512], F32, tag="mm")
                    for dc in range(8):
                        nc.tensor.matmul(
                            psv,
                            lhsT=xkt[:, dc, jt * 128:(jt + 1) * 128],
                            rhs=wv_sb[:, dc, nt * 512:(nt + 1) * 512],
                            start=(dc == 0), stop=(dc == 7),
                        )
                    vdst = v_q[:, jt, nt * 8 * 65:(nt + 1) * 8 * 65].rearrange(
                        "p (h c) -> p h c", c=65
                    )[:, :, 0:64]
                    nc.vector.tensor_add(
                        out=vdst,
                        in0=psv.rearrange("p (h c) -> p h c", c=64),
                        in1=bv_rep[:, nt * 512:(nt + 1) * 512].rearrange(
                            "p (h c) -> p h c", c=64
                        ),
                    )
            ones_view = v_q.rearrange("p j (h c) -> p j h c", c=65)[:, :, :, 64:65]
            nc.vector.memset(ones_view, 1.0)
            return v_q

        def emit_kt_slice(kT_q, q, ec):
            xkt = xkt_tiles[q]
            for half in range(2):
                psk = ps_mm.tile([128, 512], F32, tag="mm")
                for dc in range(8):
                    nc.tensor.matmul(
                        psk,
                        lhsT=wk_sb[:, dc, ec * 128:(ec + 1) * 128],
                        rhs=xkt[:, dc, half * 512:(half + 1) * 512],
                        start=(dc == 0), stop=(dc == 7),
                    )
                nc.vector.tensor_scalar(
                    out=kT_q[:, ec, half * 512:(half + 1) * 512], in0=psk,
                    scalar1=bk_sb[:, ec:ec + 1], scalar2=None, op0=Alu.add,
                )

        def emit_scores_exp(kT_q, hc):
            ets = []
            for jjp in range(4):
                pst = ps_st.tile([128, 4, NQ], F32, tag="st")
                # slot = par*2 + u: each par (concurrent row-group MMs) gets
                # its own PSUM bank; within a bank the two u-groups are
                # sequential.  Concurrent groups in ONE bank hang the HW
                # (start=True clears the bank an in-flight MM drains into).
                for u in range(2):
                    jj = jjp * 2 + u
                    for par in range(2):
                        pb = par * 64
                        nc.tensor.matmul(
                            pst[:, par * 2 + u, :],
                            lhsT=kT_q[pb:pb + 64, hc, jj * 128:(jj + 1) * 128],
                            rhs=qT[pb:pb + 64, hc, :],
                            start=True, stop=True,
                        )
                et = etp.tile([128, 4, NQ], MM_DT, tag="et")
                nc.scalar.activation(out=et, in_=pst, func=Act.Exp)
                ets.append(et)
            return ets

        def emit_attnv(v_q, ets, hc, q):
            # separate bank-padded accumulators per head (same rule)
            pso0 = ps_ot.tile([65, 512], F32, tag="ot")
            pso1 = ps_ot.tile([65, 512], F32, tag="ot")
            psos = (pso0, pso1)
            for jjp in range(4):
                for u in range(2):
                    jj = jjp * 2 + u
                    for par in range(2):
                        h = hc * 2 + par
                        nc.tensor.matmul(
                            psos[par][:, 0:NQ],
                            lhsT=v_q[:, jj, h * 65:(h + 1) * 65],
                            rhs=ets[jjp][:, par * 2 + u, :],
                            start=(jj == 0), stop=(jj == 7),
                        )
            for par in range(2):
                h = 2 * hc + par
                if q == 0:
                    nc.vector.tensor_copy(out=otacc[:, h, :],
                                          in_=psos[par][:, 0:NQ])
                else:
                    nc.vector.tensor_add(out=otacc[:, h, :],
                                         in0=otacc[:, h, :],
                                         in1=psos[par][:, 0:NQ])

        def emit_head_norm(hc):
            # O_h /= den_h for heads 2hc, 2hc+1 (den kept in otacc row 64)
            nc.vector.reciprocal(
                out=otacc[64:65, 2 * hc:2 * hc + 2, :],
                in_=otacc[64:65, 2 * hc:2 * hc + 2, :],
            )
            for k in range(2):
                h = 2 * hc + k
                psb = ps_st.tile([64, NQ], F32, tag="st")
                nc.tensor.matmul(
                    psb, lhsT=ones_t[64:65, :], rhs=otacc[64:65, h, :],
                    start=True, stop=True,
                )
                nc.vector.tensor_mul(
                    out=ot_n[:, h, :], in0=otacc[0:64, h, :], in1=psb
                )

        # ---- main quarter loop.  Quarter q: V proj, then per head-pair hc:
        # kT slice, scores+exp, (prev hc's attnv), prep of quarter q+1's
        # x^T tile hc.  Attention exp latency hides behind projection MMs. ----
        for q in range(NQTR):
            emit_input_dmas(q + 2)
            v_q = emit_v_proj(q)
            emit_stats(q + 1)
            alloc_xkt(q + 1)
            kT_q = ktp.tile([128, 8, QTR], MM_DT, tag="kt")
            pending = None  # hc awaiting attnv
            for hc in range(8):
                emit_kt_slice(kT_q, q, hc)
                ets = emit_scores_exp(kT_q, hc)
                if pending is not None:
                    emit_attnv(v_q, pending[0], pending[1], q)
                    if q == NQTR - 1:
                        emit_head_norm(pending[1])
                pending = (ets, hc)
                emit_prep_tile(q + 1, hc)
            emit_attnv(v_q, pending[0], pending[1], q)
            if q == NQTR - 1:
                emit_head_norm(pending[1])

        # ---- out = sum_h O_h @ Wout_h ----
        for ic in range(2):
            psf0 = ps_mm.tile([128, 512], F32, tag="mm")
            psf1 = ps_mm.tile([128, 512], F32, tag="mm")
            psf = [psf0, psf1]
            for h in range(16):
                for ft in range(2):
                    nc.tensor.matmul(
                        psf[ft],
                        lhsT=ot_n[:, h, ic * 128:(ic + 1) * 128],
                        rhs=wo_sb[:, h, ft * 512:(ft + 1) * 512],
                        start=(h == 0), stop=(h == 15),
                    )
            ot = outp.tile([128, D], F32, tag="outsb")
            for ft in range(2):
                nc.scalar.activation(
                    out=ot[:, ft * 512:(ft + 1) * 512], in_=psf[ft],
                    func=Act.Copy,
                )
            nc.sync.dma_start(out=out[ic * 128:(ic + 1) * 128, :], in_=ot)


_CACHED = None


def _get_program():
    global _CACHED
    if _CACHED is None:
        _CACHED = build_program()
    return _CACHED


def _mm_np():
    import ml_dtypes

    return ml_dtypes.bfloat16


def _prep_inputs(x, query, Wq, Wkv, Wout, ln_q_g, ln_q_b, ln_k_g, ln_k_b):
    scale = DH ** -0.5
    f32 = np.float32
    Wq = np.asarray(Wq, f32)
    Wkv = np.asarray(Wkv, f32)
    Wout = np.asarray(Wout, f32)
    wq_eff = (np.asarray(ln_q_g, f32)[:, None] * Wq * scale).astype(f32)
    bq_eff = (np.asarray(ln_q_b, f32) @ Wq * scale).astype(f32)
    wk_eff = (np.asarray(ln_k_g, f32)[:, None] * Wkv[:, :D]).astype(f32)
    bk_eff = (np.asarray(ln_k_b, f32) @ Wkv[:, :D]).astype(f32)
    wv_eff = (np.asarray(ln_k_g, f32)[:, None] * Wkv[:, D:]).astype(f32)
    bv_eff = (np.asarray(ln_k_b, f32) @ Wkv[:, D:]).astype(f32)
    mdt = _mm_np()
    shared = {
        "qry": np.ascontiguousarray(np.asarray(query, f32)),
        "wq": np.ascontiguousarray(wq_eff.astype(mdt)),
        "wk": np.ascontiguousarray(wk_eff.astype(mdt)),
        "wv": np.ascontiguousarray(wv_eff.astype(mdt)),
        "wo": np.ascontiguousarray(Wout.astype(mdt)),
        "bq": np.ascontiguousarray(bq_eff.reshape(8, 128).T),
        "bk": np.ascontiguousarray(bk_eff.reshape(8, 128).T),
        "bv": np.ascontiguousarray(bv_eff),
    }
    x = np.asarray(x, f32).astype(mdt)
    in_maps = [
        dict(shared, x=np.ascontiguousarray(x[i])) for i in range(NCORES)
    ]
    return in_maps


def run(trace=False, **inputs):
    from concourse.bass_utils import run_bass_kernel_spmd

    nc = _get_program()
    in_maps = _prep_inputs(**inputs)
    res = run_bass_kernel_spmd(
        nc, in_maps, core_ids=list(range(NCORES)), trace=trace
    )
    out = np.stack([res.results[i]["out"] for i in range(NCORES)], axis=0)
    return out.astype(np.float32), res.exec_time_ns


def kernel(**inputs):
    out, _ = run(trace=False, **inputs)
    return out
